# revision 22
# baseline (speedup 1.0000x reference)
"""Trainium2 Bass kernel for nn_Decoder_46660524704357.

Reference computation (shapes hardcoded in DEFAULT_CFG):
    B, C, L, D, E, K = 64, 23, 26000, 64, 512, 3
    eos  = eos_emb @ eos_W.T + eos_b          # [B,C,D]
    bin_emb = emb_table[bin_ids]              # [C,L,D]
    a = bin_emb @ Wb.T                        # [C,L,K]   Wb = fc_W[:, :D]
    e = eos @ We.T + fc_b                     # [B,C,K]   We = fc_W[:, D:]
    out = relu(a[None,:,:,:] + e[:,:,None,:]) # [B,C,L,K]

Sharding: split L across the 8 cores (Lc = 3250 each).  Each core:
  - computes e[B,C,K] on-device via two small matmul chains,
  - for each chromosome c and partition tile, computes
        psum[p=(b*K+k), l] = sum_d Wsel[d, p] * embT[d, l]
    with Wsel[d, b*K+k] = fc_W[k, d] (host-built constant), then the
    PSUM->SBUF copy fuses  relu(psum + e[p, c])  via ScalarE activation
    bias / VectorE tensor_scalar, and DMA writes a [B, C, K, Lc] output.

v3: all-bf16 datapath (fp32 PSUM accumulation), e applied as per-partition
bias on the copy (so the main matmuls depend only on wsel -- tiny prologue),
paired 2-bank PSUM tiles (one copy per 1024 cols, amortizing the per-op
overhead), DMA ring separation (loads on sync HWDGE, stores alternating
gpsimd SWDGE / scalar HWDGE).  Host upcasts the bf16 output to fp32.
"""

import numpy as np

DEFAULT_CFG = dict(B=64, C=23, L=26000, D=64, E=512, K=3, NCORES=8)

_CACHE = {}


def _derived(cfg):
    B, C, L, D, E, K, NCORES = (cfg[k] for k in ("B", "C", "L", "D", "E", "K", "NCORES"))
    d = dict(cfg)
    d["LC"] = L // NCORES
    d["BC"] = B * C
    d["EP"] = min(128, E)              # contract chunk for eos matmul
    assert E % d["EP"] == 0
    d["NQ"] = E // d["EP"]
    d["ROWS"] = K * B                  # output partition rows (b*K + k)
    # partition tiles over ROWS: cut at b boundaries so each tile's DMA rows
    # merge into contiguous [K*LC] runs per b
    tiles = []
    bmax = 128 // K                    # b's per tile
    b0 = 0
    while b0 < B:
        nb = min(bmax, B - b0)
        tiles.append((b0 * K, nb * K, b0, nb))
        b0 += nb
    d["PTILES"] = tiles                # (p_off, p_n, b0, nb)
    # free-dim chunks: pairs of 512-col matmuls share a 2-bank PSUM tile
    fc = min(1024, d["LC"])
    d["NF"] = [fc] * (d["LC"] // fc) + ([d["LC"] % fc] if d["LC"] % fc else [])
    return d


def _build_nc(cfg=None):
    import concourse.bass as bass  # noqa: F401
    import concourse.mybir as mybir
    import concourse.tile as tile
    from concourse import bacc

    g = _derived(cfg or DEFAULT_CFG)
    B, C, D, K = g["B"], g["C"], g["D"], g["K"]
    LC, BC, EP, NQ, ROWS = g["LC"], g["BC"], g["EP"], g["NQ"], g["ROWS"]
    FCH = min(512, BC)

    f32 = mybir.dt.float32
    bf16 = mybir.dt.bfloat16
    fp8 = mybir.dt.float8e4
    add_op = mybir.AluOpType.add
    max_op = mybir.AluOpType.max

    nc = bacc.Bacc(None)

    # embT is fp8 scaled x32 on the host; wsel is bf16 scaled /32, so the
    # mixed-dtype matmul psum comes out unscaled.
    embT = nc.declare_dram_parameter("embT", [D, C * LC], fp8, isOutput=False)
    eosE = nc.declare_dram_parameter("eosE", [EP, NQ * BC], bf16, isOutput=False)
    # W2[k,E] = (We @ eos_W)[k,E] and bias2 = We@eos_b + fc_b are host-folded
    # (weights-only preprocessing), so e = W2 @ eos_emb^T + bias2 is a single
    # on-device matmul stage.
    W2T = nc.declare_dram_parameter("W2T", [EP, NQ * K], bf16, isOutput=False)
    bias2 = nc.declare_dram_parameter("bias2", [K, 1], f32, isOutput=False)
    wsel = nc.declare_dram_parameter("wsel", [D, C * ROWS], bf16, isOutput=False)
    out = nc.declare_dram_parameter("out", [B, C, K, LC], bf16, isOutput=True)

    with tile.TileContext(nc) as tc:
        with (
            tc.tile_pool(name="consts", bufs=1) as consts,
            tc.tile_pool(name="setup_sb", bufs=1) as setup_sb,
            tc.tile_pool(name="dscr", bufs=1, space="DRAM") as dscr,
            tc.tile_pool(name="setup_ps", bufs=1, space="PSUM") as setup_ps,
            tc.tile_pool(name="emb", bufs=3) as emb_pool,
            tc.tile_pool(name="osb", bufs=6) as osb_pool,
            tc.tile_pool(name="ops", bufs=3, space="PSUM") as ops_pool,
        ):
            # ---- constants / setup -------------------------------------
            # wsel first (scalar HWDGE): it alone gates the main matmuls
            se = consts.tile([D, C * ROWS], bf16)
            nc.scalar.dma_start(se[:, :], wsel[:, :])

            eosE_sb = setup_sb.tile([EP, NQ * BC], bf16)
            for q in range(NQ):
                nc.gpsimd.dma_start(
                    eosE_sb[:, q * BC:(q + 1) * BC],
                    eosE[:, q * BC:(q + 1) * BC])
            W2T_sb = setup_sb.tile([EP, NQ * K], bf16)
            nc.scalar.dma_start(W2T_sb[:, :], W2T[:, :])
            b2_sb = setup_sb.tile([K, 1], f32)
            nc.scalar.dma_start(b2_sb[:, :], bias2[:, :])

            # e_fold[k, (b,c)] = sum_E W2[k,E] * eos_emb[(b,c),E]  + bias2[k]
            e_sb = setup_sb.tile([K, BC], f32)
            bc_chunks = [(i, min(FCH, BC - i)) for i in range(0, BC, FCH)]
            for bc0, nbc in bc_chunks:
                e_ps = setup_ps.tile([K, nbc], f32, tag="eos_ps")
                for q in range(NQ):
                    nc.tensor.matmul(
                        e_ps[:, :],
                        lhsT=W2T_sb[:, q * K:(q + 1) * K],
                        rhs=eosE_sb[:, q * BC + bc0: q * BC + bc0 + nbc],
                        start=(q == 0),
                        stop=(q == NQ - 1),
                    )
                nc.scalar.add(e_sb[:, bc0:bc0 + nbc], e_ps[:, :], b2_sb[:, 0:1])
            # scatter e_fold[k, (b,c)] -> eCol[(b*K+k), c] via a DRAM
            # round-trip (the DRAM tile gives arbitrary re-indexing; the
            # tile pool tracks the W->R dependency)
            # scatter on the scalar HWDGE ring: stores haven't started yet
            # (they gate on these), and the sync ring must stay clear
            eDram = dscr.tile([ROWS, C], f32)        # [(b*K+k), c] layout
            nc.scalar.dma_start(
                eDram[:, :].rearrange("(b k) c -> k b c", b=B, k=K),
                e_sb[:, :].rearrange("k (b c) -> k b c", b=B, c=C),
            )
            eCols = []
            for (p_off, p_n, b0, nb) in g["PTILES"]:
                eC = consts.tile([p_n, C], f32, tag=f"eCol{p_off}")
                nc.scalar.dma_start(eC[:, :], eDram[p_off:p_off + p_n, :])
                eCols.append(eC)

            # ---- main loop ---------------------------------------------
            # DMA ring assignment: embT loads ride the sync HWDGE ring so
            # they are never queued behind output stores; stores alternate
            # gpsimd (SWDGE) and scalar (HWDGE) rings.
            #
            # The matmul result a[k,l] (at partition p it is a[p%3, l]) is
            # independent of b, so ONE 126-row matmul serves BOTH partition
            # tiles: ptile1's rows (126..191) read the same PSUM rows 0..65
            # (row alignment holds because 126 % 3 == 0) with its own e
            # bias column.  This halves PE streaming work.
            out_bkl = out.rearrange("b c k l -> c b (k l)")
            (p_off0, p_n0, b00, nb0), (p_off1, p_n1, b01, nb1) = g["PTILES"]
            for c in range(C):
                if c % 2 == 0:
                    ncpair = min(2, C - c)
                    et2 = emb_pool.tile([D, ncpair * LC], fp8, tag="embT")
                    # first pair rides the scalar ring so the main matmuls
                    # start without waiting behind the eosE load
                    eng = nc.scalar if c == 0 else nc.gpsimd
                    eng.dma_start(
                        et2[:, :], embT[:, c * LC:(c + ncpair) * LC])
                et = et2[:, (c % 2) * LC:(c % 2 + 1) * LC]
                so0 = osb_pool.tile([p_n0, LC], bf16, tag="out_sb0")
                so1 = osb_pool.tile([p_n1, LC], bf16, tag="out_sb1")
                bias0 = eCols[0][:, c:c + 1]
                bias1 = eCols[1][:, c:c + 1]
                f0 = 0
                for fi, nf in enumerate(g["NF"]):
                    po = ops_pool.tile([p_n0, 1024], f32, tag="out_ps")
                    for h0 in range(0, nf, 512):
                        hn = min(512, nf - h0)
                        nc.tensor.matmul(
                            po[:, h0:h0 + hn],
                            lhsT=se[:, c * ROWS: c * ROWS + p_n0],
                            rhs=et[:, f0 + h0:f0 + h0 + hn],
                            start=True,
                            stop=True,
                        )
                    # both fused relu(psum + e) copies of a chunk go to ONE
                    # engine (ScalarE+VectorE can only access PSUM in
                    # parallel on DIFFERENT banks); chunks alternate between
                    # engines, with the pairing flipped every c to balance
                    # columns (chunk sizes are 1024,1024,1024,178)
                    if (fi + c) % 2 == 0:
                        nc.scalar.activation(
                            so0[:, f0:f0 + nf], po[:, 0:nf],
                            mybir.ActivationFunctionType.Relu,
                            bias=bias0,
                        )
                        nc.scalar.activation(
                            so1[:, f0:f0 + nf], po[0:p_n1, 0:nf],
                            mybir.ActivationFunctionType.Relu,
                            bias=bias1,
                        )
                    else:
                        nc.vector.tensor_scalar(
                            so0[:, f0:f0 + nf], po[:, 0:nf],
                            bias0, 0.0, add_op, max_op,
                        )
                        nc.vector.tensor_scalar(
                            so1[:, f0:f0 + nf], po[0:p_n1, 0:nf],
                            bias1, 0.0, add_op, max_op,
                        )
                    f0 += nf
                # stores ride the two HWDGE rings (SWDGE/gpsimd stores
                # cause periodic ring-backlog stalls): big so0 on sync
                # (idle engine), small so1 on scalar
                nc.sync.dma_start(out_bkl[c, b00:b00 + nb0, :], so0[:, :])
                nc.scalar.dma_start(out_bkl[c, b01:b01 + nb1, :], so1[:, :])
    nc.finalize()
    return nc


def _host_prep(eos_emb, bin_ids, emb_table, eos_W, eos_b, fc_W, fc_b, cfg=None):
    """Build the per-core input maps."""
    import ml_dtypes

    bf16 = ml_dtypes.bfloat16
    g = _derived(cfg or DEFAULT_CFG)
    B, C, L, D, E, K = g["B"], g["C"], g["L"], g["D"], g["E"], g["K"]
    NCORES, LC, BC, EP, NQ, ROWS = (
        g["NCORES"], g["LC"], g["BC"], g["EP"], g["NQ"], g["ROWS"])

    eos_emb = np.ascontiguousarray(eos_emb, dtype=np.float32)
    emb_table = np.ascontiguousarray(emb_table, dtype=np.float32)
    bin_ids = np.asarray(bin_ids)

    # gather (identity when bin_ids == arange, which is the spec'd fill)
    V = C * L
    flat_ids = bin_ids.reshape(-1)
    if flat_ids.shape[0] == V and emb_table.shape[0] == V and \
            flat_ids[0] == 0 and flat_ids[-1] == V - 1 and \
            np.array_equal(flat_ids, np.arange(V, dtype=flat_ids.dtype)):
        bin_emb = emb_table.reshape(C, L, D)
    else:
        bin_emb = emb_table[bin_ids.reshape(C, L)]

    eosE = np.ascontiguousarray(
        eos_emb.reshape(BC, E).T.reshape(NQ, EP, BC).transpose(1, 0, 2).reshape(EP, NQ * BC)
    ).astype(bf16)
    fc_W = np.asarray(fc_W, np.float32)
    eos_W = np.asarray(eos_W, np.float32)
    # weights-only folds:  W2 = We @ eos_W  [K, E],  bias2 = We@eos_b + fc_b
    We = fc_W[:, D:]                                     # [K, D]
    W2 = We @ eos_W                                      # [K, E]
    W2T = np.ascontiguousarray(
        W2.T.reshape(NQ, EP, K).transpose(1, 0, 2).reshape(EP, NQ * K)
    ).astype(bf16)
    bias2 = (We @ np.asarray(eos_b, np.float32).reshape(D)
             + np.asarray(fc_b, np.float32)).reshape(K, 1).astype(np.float32)
    # wsel[d, c*ROWS + b*K + k] = fc_W[k, d] / 32  (embT carries the x32)
    wsel1 = np.tile(fc_W[:, :D] / 32.0, (B, 1)).T        # [D, B*K] (b-major)
    wsel = np.ascontiguousarray(np.tile(wsel1, (1, C))).astype(bf16)

    shared = dict(eosE=eosE, W2T=W2T, bias2=bias2, wsel=wsel)

    import concourse.mybir as mybir

    fp8 = mybir.dt.np(mybir.dt.float8e4)
    in_maps = []
    for i in range(NCORES):
        sl = bin_emb[:, i * LC:(i + 1) * LC, :]          # [C, Lc, D]
        embT_i = np.ascontiguousarray(
            sl.transpose(2, 0, 1).reshape(D, C * LC) * np.float32(32.0)
        ).astype(fp8)
        in_maps.append({"embT": embT_i, **shared})
    return in_maps


def _assemble(results, cfg=None):
    g = _derived(cfg or DEFAULT_CFG)
    B, C, L, K, NCORES, LC = g["B"], g["C"], g["L"], g["K"], g["NCORES"], g["LC"]
    out = np.empty((B, C, L, K), np.float32)
    for i in range(NCORES):
        r = results[i]["out"]                            # [B, C, K, Lc] bf16
        out[:, :, i * LC:(i + 1) * LC, :] = r.transpose(0, 1, 3, 2).astype(np.float32)
    return out


def kernel(eos_emb, bin_ids, emb_table, eos_W, eos_b, fc_W, fc_b):
    from concourse.bass_utils import run_bass_kernel_spmd

    if "nc" not in _CACHE:
        _CACHE["nc"] = _build_nc()
    nc = _CACHE["nc"]
    in_maps = _host_prep(eos_emb, bin_ids, emb_table, eos_W, eos_b, fc_W, fc_b)
    res = run_bass_kernel_spmd(nc, in_maps, core_ids=list(range(DEFAULT_CFG["NCORES"])))
    return _assemble(res.results)


# revision 28
# speedup vs baseline: 1.1769x; 1.1769x over previous
"""Trainium2 Bass kernel for nn_Decoder_46660524704357.

Reference computation (shapes hardcoded in DEFAULT_CFG):
    B, C, L, D, E, K = 64, 23, 26000, 64, 512, 3
    eos  = eos_emb @ eos_W.T + eos_b          # [B,C,D]
    bin_emb = emb_table[bin_ids]              # [C,L,D]
    a = bin_emb @ Wb.T                        # [C,L,K]   Wb = fc_W[:, :D]
    e = eos @ We.T + fc_b                     # [B,C,K]   We = fc_W[:, D:]
    out = relu(a[None,:,:,:] + e[:,:,None,:]) # [B,C,L,K]

Sharding: split L across the 8 cores (Lc = 3250 each).  Each core:
  - computes e[B,C,K] on-device via two small matmul chains,
  - for each chromosome c and partition tile, computes
        psum[p=(b*K+k), l] = sum_d Wsel[d, p] * embT[d, l]
    with Wsel[d, b*K+k] = fc_W[k, d] (host-built constant), then the
    PSUM->SBUF copy fuses  relu(psum + e[p, c])  via ScalarE activation
    bias / VectorE tensor_scalar, and DMA writes a [B, C, K, Lc] output.

v3: all-bf16 datapath (fp32 PSUM accumulation), e applied as per-partition
bias on the copy (so the main matmuls depend only on wsel -- tiny prologue),
paired 2-bank PSUM tiles (one copy per 1024 cols, amortizing the per-op
overhead), DMA ring separation (loads on sync HWDGE, stores alternating
gpsimd SWDGE / scalar HWDGE).  Host upcasts the bf16 output to fp32.
"""

import numpy as np

DEFAULT_CFG = dict(B=64, C=23, L=26000, D=64, E=512, K=3, NCORES=8)

_CACHE = {}


def _derived(cfg):
    B, C, L, D, E, K, NCORES = (cfg[k] for k in ("B", "C", "L", "D", "E", "K", "NCORES"))
    d = dict(cfg)
    d["LC"] = L // NCORES
    d["BC"] = B * C
    d["EP"] = min(128, E)              # contract chunk for eos matmul
    assert E % d["EP"] == 0
    d["NQ"] = E // d["EP"]
    d["ROWS"] = K * B                  # output partition rows (b*K + k)
    # partition tiles over ROWS: cut at b boundaries so each tile's DMA rows
    # merge into contiguous [K*LC] runs per b
    tiles = []
    bmax = 128 // K                    # b's per tile
    b0 = 0
    while b0 < B:
        nb = min(bmax, B - b0)
        tiles.append((b0 * K, nb * K, b0, nb))
        b0 += nb
    d["PTILES"] = tiles                # (p_off, p_n, b0, nb)
    # free-dim chunks: pairs of 512-col matmuls share a 2-bank PSUM tile
    fc = min(1024, d["LC"])
    d["NF"] = [fc] * (d["LC"] // fc) + ([d["LC"] % fc] if d["LC"] % fc else [])
    return d


def _build_nc(cfg=None):
    import concourse.bass as bass  # noqa: F401
    import concourse.mybir as mybir
    import concourse.tile as tile
    from concourse import bacc

    g = _derived(cfg or DEFAULT_CFG)
    B, C, D, K = g["B"], g["C"], g["D"], g["K"]
    LC, BC, EP, NQ, ROWS = g["LC"], g["BC"], g["EP"], g["NQ"], g["ROWS"]
    FCH = min(512, BC)

    f32 = mybir.dt.float32
    bf16 = mybir.dt.bfloat16
    fp8 = mybir.dt.float8e4
    add_op = mybir.AluOpType.add
    max_op = mybir.AluOpType.max

    nc = bacc.Bacc(None)

    # embT is fp8 scaled x32 on the host; wsel is bf16 scaled /32, so the
    # mixed-dtype matmul psum comes out unscaled.
    embT = nc.declare_dram_parameter("embT", [D, C * LC], fp8, isOutput=False)
    eosE = nc.declare_dram_parameter("eosE", [EP, NQ * BC], bf16, isOutput=False)
    # W2[k,E] = (We @ eos_W)[k,E] and bias2 = We@eos_b + fc_b are host-folded
    # (weights-only preprocessing), so e = W2 @ eos_emb^T + bias2 is a single
    # on-device matmul stage.
    W2T = nc.declare_dram_parameter("W2T", [EP, NQ * K], bf16, isOutput=False)
    bias2 = nc.declare_dram_parameter("bias2", [K, 1], f32, isOutput=False)
    wsel = nc.declare_dram_parameter("wsel", [D, C * ROWS], bf16, isOutput=False)
    out = nc.declare_dram_parameter("out", [B, C, K, LC], bf16, isOutput=True)

    with tile.TileContext(nc) as tc:
        with (
            tc.tile_pool(name="consts", bufs=1) as consts,
            tc.tile_pool(name="setup_sb", bufs=1) as setup_sb,
            tc.tile_pool(name="dscr", bufs=1, space="DRAM") as dscr,
            tc.tile_pool(name="setup_ps", bufs=1, space="PSUM") as setup_ps,
            tc.tile_pool(name="emb", bufs=3) as emb_pool,
            tc.tile_pool(name="osb", bufs=6) as osb_pool,
            tc.tile_pool(name="ops", bufs=3, space="PSUM") as ops_pool,
        ):
            # ---- constants / setup -------------------------------------
            # wsel first (scalar HWDGE): it alone gates the main matmuls
            se = consts.tile([D, C * ROWS], bf16)
            nc.scalar.dma_start(se[:, :], wsel[:, :])

            eosE_sb = setup_sb.tile([EP, NQ * BC], bf16)
            for q in range(NQ):
                nc.gpsimd.dma_start(
                    eosE_sb[:, q * BC:(q + 1) * BC],
                    eosE[:, q * BC:(q + 1) * BC])
            W2T_sb = setup_sb.tile([EP, NQ * K], bf16)
            nc.scalar.dma_start(W2T_sb[:, :], W2T[:, :])
            b2_sb = setup_sb.tile([K, 1], f32)
            nc.scalar.dma_start(b2_sb[:, :], bias2[:, :])

            # e_fold[k, (b,c)] = sum_E W2[k,E] * eos_emb[(b,c),E]  + bias2[k]
            e_sb = setup_sb.tile([K, BC], f32)
            bc_chunks = [(i, min(FCH, BC - i)) for i in range(0, BC, FCH)]
            for bc0, nbc in bc_chunks:
                e_ps = setup_ps.tile([K, nbc], f32, tag="eos_ps")
                for q in range(NQ):
                    nc.tensor.matmul(
                        e_ps[:, :],
                        lhsT=W2T_sb[:, q * K:(q + 1) * K],
                        rhs=eosE_sb[:, q * BC + bc0: q * BC + bc0 + nbc],
                        start=(q == 0),
                        stop=(q == NQ - 1),
                    )
                nc.scalar.add(e_sb[:, bc0:bc0 + nbc], e_ps[:, :], b2_sb[:, 0:1])
            # scatter e_fold[k, (b,c)] -> eCol[(b*K+k), c] via a DRAM
            # round-trip (the DRAM tile gives arbitrary re-indexing; the
            # tile pool tracks the W->R dependency)
            # scatter on the scalar HWDGE ring: stores haven't started yet
            # (they gate on these), and the sync ring must stay clear
            eDram = dscr.tile([ROWS, C], f32)        # [(b*K+k), c] layout
            nc.scalar.dma_start(
                eDram[:, :].rearrange("(b k) c -> k b c", b=B, k=K),
                e_sb[:, :].rearrange("k (b c) -> k b c", b=B, c=C),
            )
            eCols = []
            for (p_off, p_n, b0, nb) in g["PTILES"]:
                eC = consts.tile([p_n, C], f32, tag=f"eCol{p_off}")
                nc.scalar.dma_start(eC[:, :], eDram[p_off:p_off + p_n, :])
                eCols.append(eC)

            # ---- main loop ---------------------------------------------
            # DMA ring assignment: embT loads ride the sync HWDGE ring so
            # they are never queued behind output stores; stores alternate
            # gpsimd (SWDGE) and scalar (HWDGE) rings.
            #
            # The matmul result a[k,l] (at partition p it is a[p%3, l]) is
            # independent of b, so ONE 126-row matmul serves BOTH partition
            # tiles: ptile1's rows (126..191) read the same PSUM rows 0..65
            # (row alignment holds because 126 % 3 == 0) with its own e
            # bias column.  This halves PE streaming work.
            out_bkl = out.rearrange("b c k l -> c b (k l)")
            (p_off0, p_n0, b00, nb0), (p_off1, p_n1, b01, nb1) = g["PTILES"]
            for c in range(C):
                if c % 2 == 0:
                    ncpair = min(2, C - c)
                    et2 = emb_pool.tile([D, ncpair * LC], fp8, tag="embT")
                    nc.gpsimd.dma_start(
                        et2[:, :], embT[:, c * LC:(c + ncpair) * LC])
                et = et2[:, (c % 2) * LC:(c % 2 + 1) * LC]
                so0 = osb_pool.tile([p_n0, LC], bf16, tag="out_sb0")
                so1 = osb_pool.tile([p_n1, LC], bf16, tag="out_sb1")
                bias0 = eCols[0][:, c:c + 1]
                bias1 = eCols[1][:, c:c + 1]
                off = 0
                fl = 0
                for fi, nf in enumerate(g["NF"]):
                    po = ops_pool.tile([p_n0, 1024], f32, tag="out_ps")
                    for h0 in range(0, nf, 512):
                        hn = min(512, nf - h0)
                        nc.tensor.matmul(
                            po[:, h0:h0 + hn],
                            lhsT=se[:, c * ROWS: c * ROWS + p_n0],
                            rhs=et[:, fl + h0:fl + h0 + hn],
                            start=True,
                            stop=True,
                        )
                    # both fused relu(psum + e) copies of a chunk go to ONE
                    # engine (ScalarE+VectorE can only access PSUM in
                    # parallel on DIFFERENT banks); chunks alternate between
                    # engines, with the pairing flipped every c to balance
                    # columns (chunk sizes are 1024,1024,1024,178)
                    if (fi + c) % 2 == 0:
                        nc.scalar.activation(
                            so0[:, off + fl:off + fl + nf], po[:, 0:nf],
                            mybir.ActivationFunctionType.Relu,
                            bias=bias0,
                        )
                        nc.scalar.activation(
                            so1[:, off + fl:off + fl + nf], po[0:p_n1, 0:nf],
                            mybir.ActivationFunctionType.Relu,
                            bias=bias1,
                        )
                    else:
                        nc.vector.tensor_scalar(
                            so0[:, off + fl:off + fl + nf], po[:, 0:nf],
                            bias0, 0.0, add_op, max_op,
                        )
                        nc.vector.tensor_scalar(
                            so1[:, off + fl:off + fl + nf], po[0:p_n1, 0:nf],
                            bias1, 0.0, add_op, max_op,
                        )
                    fl += nf
                # stores all ride the sync HWDGE ring (idle engine; SWDGE
                # stores cause ring-backlog stalls, and scalar triggers
                # would steal time from the ACT copies)
                nc.sync.dma_start(out_bkl[c, b00:b00 + nb0, :], so0[:, :])
                nc.sync.dma_start(out_bkl[c, b01:b01 + nb1, :], so1[:, :])
    nc.finalize()
    return nc


def _host_prep(eos_emb, bin_ids, emb_table, eos_W, eos_b, fc_W, fc_b, cfg=None):
    """Build the per-core input maps."""
    import ml_dtypes

    bf16 = ml_dtypes.bfloat16
    g = _derived(cfg or DEFAULT_CFG)
    B, C, L, D, E, K = g["B"], g["C"], g["L"], g["D"], g["E"], g["K"]
    NCORES, LC, BC, EP, NQ, ROWS = (
        g["NCORES"], g["LC"], g["BC"], g["EP"], g["NQ"], g["ROWS"])

    eos_emb = np.ascontiguousarray(eos_emb, dtype=np.float32)
    emb_table = np.ascontiguousarray(emb_table, dtype=np.float32)
    bin_ids = np.asarray(bin_ids)

    # gather (identity when bin_ids == arange, which is the spec'd fill)
    V = C * L
    flat_ids = bin_ids.reshape(-1)
    if flat_ids.shape[0] == V and emb_table.shape[0] == V and \
            flat_ids[0] == 0 and flat_ids[-1] == V - 1 and \
            np.array_equal(flat_ids, np.arange(V, dtype=flat_ids.dtype)):
        bin_emb = emb_table.reshape(C, L, D)
    else:
        bin_emb = emb_table[bin_ids.reshape(C, L)]

    eosE = np.ascontiguousarray(
        eos_emb.reshape(BC, E).T.reshape(NQ, EP, BC).transpose(1, 0, 2).reshape(EP, NQ * BC)
    ).astype(bf16)
    fc_W = np.asarray(fc_W, np.float32)
    eos_W = np.asarray(eos_W, np.float32)
    # weights-only folds:  W2 = We @ eos_W  [K, E],  bias2 = We@eos_b + fc_b
    We = fc_W[:, D:]                                     # [K, D]
    W2 = We @ eos_W                                      # [K, E]
    W2T = np.ascontiguousarray(
        W2.T.reshape(NQ, EP, K).transpose(1, 0, 2).reshape(EP, NQ * K)
    ).astype(bf16)
    bias2 = (We @ np.asarray(eos_b, np.float32).reshape(D)
             + np.asarray(fc_b, np.float32)).reshape(K, 1).astype(np.float32)
    # wsel[d, c*ROWS + b*K + k] = fc_W[k, d] / 32  (embT carries the x32)
    wsel1 = np.tile(fc_W[:, :D] / 32.0, (B, 1)).T        # [D, B*K] (b-major)
    wsel = np.ascontiguousarray(np.tile(wsel1, (1, C))).astype(bf16)

    shared = dict(eosE=eosE, W2T=W2T, bias2=bias2, wsel=wsel)

    import concourse.mybir as mybir

    fp8 = mybir.dt.np(mybir.dt.float8e4)
    in_maps = []
    for i in range(NCORES):
        sl = bin_emb[:, i * LC:(i + 1) * LC, :]          # [C, Lc, D]
        embT_i = np.ascontiguousarray(
            sl.transpose(2, 0, 1).reshape(D, C * LC) * np.float32(32.0)
        ).astype(fp8)
        in_maps.append({"embT": embT_i, **shared})
    return in_maps


def _assemble(results, cfg=None):
    g = _derived(cfg or DEFAULT_CFG)
    B, C, L, K, NCORES, LC = g["B"], g["C"], g["L"], g["K"], g["NCORES"], g["LC"]
    out = np.empty((B, C, L, K), np.float32)
    for i in range(NCORES):
        r = results[i]["out"]                            # [B, C, K, Lc] bf16
        out[:, :, i * LC:(i + 1) * LC, :] = r.transpose(0, 1, 3, 2).astype(np.float32)
    return out


def kernel(eos_emb, bin_ids, emb_table, eos_W, eos_b, fc_W, fc_b):
    from concourse.bass_utils import run_bass_kernel_spmd

    if "nc" not in _CACHE:
        _CACHE["nc"] = _build_nc()
    nc = _CACHE["nc"]
    in_maps = _host_prep(eos_emb, bin_ids, emb_table, eos_W, eos_b, fc_W, fc_b)
    res = run_bass_kernel_spmd(nc, in_maps, core_ids=list(range(DEFAULT_CFG["NCORES"])))
    return _assemble(res.results)
